# revision 30
# baseline (speedup 1.0000x reference)
"""BlockGrouper (MoE routing dispatch) Trainium2 kernel — raw bass.

Semantics (from the reference): each token n in sample b belongs to group
g = argmax(block_onehot[b, n]); its slot within the group is its rank
among same-group tokens in token order.  With the balanced one-hot
routing, the output [B, G, cap, D] is a pure row-permutation of
x [B, N, D].

Sharding: data-parallel over B across the 8 NeuronCores (one sample per
core); each core moves 16 MiB in + 16 MiB out.

v3 vs v1 (baseline): the data path scatters with plain-write dynamic
indirect DMAs (InstDMACopy, cce_op=bypass) instead of dma_scatter_add.
The scatter-add read-modify-wrote the 16 MiB output (48.5 MiB total HBM
traffic per core, measured 161 GB/s payload in the scatter phase); the
pure write drops the RMW read (HW profile confirms hbm_read_bytes is
just x+oh).  The indirect DMA takes int32 row offsets directly from
SBUF, so the whole int16 fold stage (8 repsel matmuls + strided cast)
of v1 is gone; the index pipeline ends at a single f32->i32 cast.

HW-verified ucode constraints for the indirect DMA (micro-benched):
  - exactly ONE offset per partition per call, offsets [128, 1] int32,
    payload 2D [128, elem]; multi-column offset APs corrupt addresses
    (the ucode mis-scales the dest stride and interleaves packets);
  - the `queue` attribute is ignored: all generic InstDMACopy descs go
    to SWDGE queue 0 (only the custom *Ant instructions honor
    queue_num);
  - coef is in elements of the out dtype (512 here), matching the sim.
The data phase is therefore 64 calls x 128 rows.  Per-call cost on the
Pool sequencer (~1.6 us: ~1 us ucode fixed + 128 descs + offset fetch)
makes the scatter phase call-bound at ~103 us, slightly above the
34 MiB/core HBM roofline (~95 us).  Measured: ~117-121 us vs ~138-148 us
for the v1 baseline, rel err 0.0.

Per-core program (N=8192, G=16, D=512, cap=512, P=128, C=64; token n
lives at partition p = n // 64, column c = n % 64):
  1. Index pipeline: tot[p, g] = per-partition group counts (one DVE
     reduce); PE computes the carry a_ps[p, g] = (# tokens of g before
     partition p) + g*cap - 1 via one strict-upper-triangular-ones
     matmul plus a const-row matmul; 16 strided tensor_tensor_scan ops
     (initial=a_ps[:, g]) then produce dest directly per group;
     oh*scan, reduce over g, cast to int32.  prod_j is issued after
     scan_{j+2} (and the reduce/cast halves after a spacer) so every
     same-engine RAW pair has >= 2 instructions of pipeline distance.
  2. Data path: 8 x-chunk loads (p-major, 16 KiB contiguous per
     partition, each chunk split across the SP and ACT HWDGE rings,
     per-chunk completion semaphores so out-of-order ring completions
     cannot release a consumer early) and 64 single-column indirect
     scatter-writes into the output.  A dummy scatter at t=0 pulls any
     lazy LOAD_LIB off the critical path.
"""


import numpy as np

B, N, G, D = 8, 8192, 16, 512
CAP = N // G
P = 128
C = N // P
NCORES = 8
# x-load chunks in token-columns: small first chunks so the first scatter
# columns unblock early, bigger later ones to amortize DMA count
CHUNK_COLS = [2, 2, 4, 8, 16, 32]
CHUNKS = []
_c = 0
for _w in CHUNK_COLS:
    CHUNKS.append((_c, _c + _w))
    _c += _w
assert _c == C
NCHUNK = len(CHUNKS)

_cached = None


def _indirect_scatter_write(nc, out_ap, offset_ap, in_ap, queue_name):
    """nc.gpsimd.indirect_dma_start(out, offset(axis 0), in_, bypass) with a
    parameterized SWDGE queue name (the stock method pins qPoolDynamic)."""
    import concourse.mybir as mybir

    eng = nc.gpsimd
    out_l = eng.lower_ap_dma(out_ap, for_indirect_dma=True)
    in_l = eng.lower_ap_dma(in_ap, for_indirect_dma=True)
    assert len(out_l) == 1 and len(in_l) == 1
    off_l = eng.lower_ap_dma(offset_ap)
    assert len(off_l) == 1
    in_l.append(off_l[0])

    coef = out_ap.shape[1]  # elements per row of the indirect'd axis 0
    out_l[0].dynamic_ap_info = mybir.DynamicAccessPatternInfo(
        c=0,
        actual_ap=in_ap.ap,
        indirect_dim_max_index=out_ap.shape[0],
        offset_expr=[
            mybir.DynamicAccessPatternOffsetExpr(
                coef=coef,
                aff_expr=mybir.DynamicAccessPatternOffsetExprAffExpr(
                    kind="IndirectArgId", arg_id=1
                ),
            )
        ],
    )
    return eng.add_instruction(
        mybir.InstDMACopy(
            name=nc.get_next_instruction_name(),
            queue=queue_name,
            mode="Copy",
            ins=in_l,
            outs=out_l,
            oob_is_err=False,
            cce_op=mybir.AluOpType.bypass,
        )
    )


def _build():
    import contextlib

    import concourse.bass as bass
    import concourse.bacc as bacc
    import concourse.mybir as mybir

    f32 = mybir.dt.float32
    i32 = mybir.dt.int32
    i16 = mybir.dt.int16

    nc = bacc.Bacc("TRN2", target_bir_lowering=False, debug=False,
                   num_devices=NCORES, num_swdge_queues=1,
                   detect_race_conditions=False)
    x_d = nc.dram_tensor("x", [N, D], f32, kind="ExternalInput")
    oh_d = nc.dram_tensor("oh", [N, G], f32, kind="ExternalInput")
    cst_big_d = nc.dram_tensor("cst_big", [P, P], f32,
                               kind="ExternalInput")
    cst_row_d = nc.dram_tensor("cst_row", [1, P + G], f32,
                               kind="ExternalInput")
    out_d = nc.dram_tensor("out", [N, D], f32, kind="ExternalOutput")
    # tiny scratch target for the t=0 dummy scatter that preloads any lazy
    # dynamic-DMA library off the critical path
    dummy_d = nc.dram_tensor("lib_warm", [16, 64], f32, kind="ExternalOutput")

    with (
        nc.sbuf_tensor("cst_big_t", [P, P], f32) as cst_big_t,
        nc.sbuf_tensor("cst_row_t", [1, P + G], f32) as cst_row_t,
        nc.sbuf_tensor("oh_t", [P, C * G], f32) as oh_t,
        nc.sbuf_tensor("tot_t", [P, G], f32) as tot_t,
        nc.sbuf_tensor("scan_t", [P, C * G], f32) as scan_t,
        nc.sbuf_tensor("prod_t", [P, C * G], f32) as prod_t,
        nc.sbuf_tensor("dest_f", [P, C], f32) as dest_f,
        nc.sbuf_tensor("dest_i", [P, C], i32) as dest_i,
        nc.sbuf_tensor("xt", [P, C * D], f32) as xt,
        nc.psum_tensor("a_ps", [P, G], f32) as a_ps,
        contextlib.ExitStack() as stack,
        nc.semaphore("s_const") as s_const,
        nc.semaphore("s_oh") as s_oh,
        nc.semaphore("s_dve") as s_dve,
        nc.semaphore("s_pe") as s_pe,
        nc.semaphore("s_warm") as s_warm,
    ):
        # per-ring x-load sems: chunk k fully arrived iff both rings have
        # delivered their k-th half (each ring is FIFO)
        s_xs = stack.enter_context(nc.semaphore("s_xs"))
        s_xc = stack.enter_context(nc.semaphore("s_xc"))
        s_sq0 = stack.enter_context(nc.semaphore("s_sq0"))
        dummy_idx = stack.enter_context(
            nc.sbuf_tensor("dummy_idx", [P, 1], i32))
        dummy_pay = stack.enter_context(
            nc.sbuf_tensor("dummy_pay", [P, 64], f32))
        su_t = cst_big_t[:, 0:P]
        ones_t = cst_row_t[:, 0:P]
        cst_t = cst_row_t[:, P:P + G]

        # ---------------- plain DMAs ----------------
        # oh gates the whole index pipeline: split it across BOTH HWDGE
        # rings (SP + ACT) so it lands ~1.3us earlier; constants follow on
        # the ACT ring.
        oh3 = oh_d[:].rearrange("(p c) g -> p c g", p=P)
        oht3 = oh_t[:].rearrange("p (c g) -> p c g", g=G)
        q4 = C // 4
        for qi in range(2):
            nc.sync.dma_start(
                out=oht3[:, qi * q4:(qi + 1) * q4, :],
                in_=oh3[:, qi * q4:(qi + 1) * q4, :]).then_inc(s_oh, 16)
            nc.scalar.dma_start(
                out=oht3[:, (qi + 2) * q4:(qi + 3) * q4, :],
                in_=oh3[:, (qi + 2) * q4:(qi + 3) * q4, :]).then_inc(
                s_oh, 16)
        nc.scalar.dma_start(out=cst_big_t[:], in_=cst_big_d[:]).then_inc(
            s_const, 16)
        nc.scalar.dma_start(out=cst_row_t[:], in_=cst_row_d[:]).then_inc(
            s_const, 16)
        # p-major: the scatter for column c carries x rows for tokens
        # p * 64 + c.  Chunk sizes ramp up (small first chunks) so the
        # first scatter columns unblock as early as possible; halves of
        # each chunk go to the two HWDGE rings.
        x3 = x_d[:].rearrange("(p c) d -> p c d", p=P)
        xto = xt[:].rearrange("p (c d) -> p c d", d=D)
        for k, (c0, c1) in enumerate(CHUNKS):
            h = (c1 - c0) // 2
            nc.sync.dma_start(
                out=xto[:, c0:c0 + h, :],
                in_=x3[:, c0:c0 + h, :]).then_inc(s_xs, 16)
            nc.scalar.dma_start(
                out=xto[:, c0 + h:c1, :],
                in_=x3[:, c0 + h:c1, :]).then_inc(s_xc, 16)

        # ---------------- DVE ----------------
        # tot[p, g] = number of group-g tokens in partition p; the PE turns
        # it into the carry a_ps[p, g] = (tokens of g before partition p)
        # + g*cap - 1.  The scans then start from that carry directly, so
        # scan_g[p, c] == dest for group-g tokens; prod*reduce collapses
        # over g.  Same-engine RAW pairs are kept >= 2 instructions apart
        # (DVE does not interlock close RAW hazards).
        nc.vector.wait_ge(s_oh, 64)
        # tot[p, g] = sum_c oh[p, c, g]: contiguous pre-add of the column
        # halves, then a strided reduce over the remaining 32 columns
        nc.vector.tensor_tensor(
            out=scan_t[:, 0:C * G // 2], in0=oh_t[:, 0:C * G // 2],
            in1=oh_t[:, C * G // 2:C * G], op=mybir.AluOpType.add)
        nc.vector.tensor_reduce(
            out=tot_t[:],
            in_=scan_t[:, 0:C * G // 2].rearrange("p (c g) -> p g c", g=G),
            axis=mybir.AxisListType.X,
            op=mybir.AluOpType.add).then_inc(s_dve, 1)

        def scan_g(g):
            ins = nc.vector.tensor_tensor_scan(
                out=scan_t[:, g::G], data0=oh_t[:, g::G],
                data1=oh_t[:, g::G], initial=a_ps[:, g:g + 1],
                op0=mybir.AluOpType.add, op1=mybir.AluOpType.bypass)
            return ins

        def prod_g(g):
            nc.vector.tensor_tensor(
                out=prod_t[:, g::G], in0=oh_t[:, g::G],
                in1=scan_t[:, g::G], op=mybir.AluOpType.mult)

        nc.vector.wait_ge(s_pe, 1)
        for g in range(G):
            scan_g(g)
        # DVE issue rate (~220-270ns/op) dominates op cost at these sizes,
        # so ONE contiguous 1024-elem prod beats 16 strided ones.  prod_g(0)
        # spaces the big prod >= 2 ops from scan_15; prod_g(1) spaces the
        # reduce >= 2 ops from the big prod (its own distance-1 rewrite is
        # benign: it duplicates values the big prod already wrote).
        prod_g(0)
        nc.vector.tensor_tensor(
            out=prod_t[:], in0=oh_t[:], in1=scan_t[:],
            op=mybir.AluOpType.mult)
        prod_g(1)
        h = C // 2
        for i in range(2):
            nc.vector.tensor_reduce(
                out=dest_f[:, i * h:(i + 1) * h],
                in_=prod_t[:, i * h * G:(i + 1) * h * G].rearrange(
                    "p (c g) -> p c g", g=G),
                axis=mybir.AxisListType.X,
                op=mybir.AluOpType.add)
        # cast halves separately so the scatter can start on columns 0:32
        # while the second half finishes (s_dve=2 first half, 3 second)
        nc.vector.tensor_copy(out=dest_i[:, 0:h],
                              in_=dest_f[:, 0:h]).then_inc(s_dve, 1)
        nc.vector.tensor_copy(out=dest_i[:, h:C],
                              in_=dest_f[:, h:C]).then_inc(s_dve, 1)

        # ---------------- PE ----------------
        nc.tensor.wait_ge(s_const, 32)
        nc.tensor.wait_ge(s_dve, 1)
        nc.tensor.matmul(out=a_ps[:], lhsT=su_t, rhs=tot_t[:],
                         start=True, stop=False)
        nc.tensor.matmul(out=a_ps[:], lhsT=ones_t, rhs=cst_t,
                         start=False, stop=True).then_inc(s_pe, 1)

        # ---------------- Pool: indirect scatter-writes ----------------
        qname = ["qPoolDynamic", "qPoolDynamic1", "qPoolDynamic2",
                 "qPoolDynamic3"]
        # dummy scatter at t=0 warms the dynamic-DMA path.  The ucode only
        # supports one offset per partition and a 2D [128, D] payload per
        # call, so the main loop is one call per token column: 64 calls x
        # 128 rows of 2 KiB (~1.1 us of Pool desc-gen each, measured).
        # v4: no dma_scatter_add columns at all — the Ant calls cost
        # ~5.7 us each of serial Pool time plus a ~12 us LOAD_LIB stall in
        # front of the generic calls, and their CCE RMW re-reads the
        # output (4 MiB extra HBM traffic).  All-generic is both cheaper
        # on the Pool queue and lighter on the bus.
        nc.gpsimd.memset(dummy_idx[:], 0).then_inc(s_warm, 1)
        nc.gpsimd.memset(dummy_pay[:], 0).then_inc(s_warm, 1)
        nc.gpsimd.wait_ge(s_warm, 2)
        _indirect_scatter_write(
            nc, dummy_d[:], dummy_idx[:], dummy_pay[:],
            qname[0]).then_inc(s_sq0, 16)

        chunk_start = {c0: k for k, (c0, c1) in enumerate(CHUNKS)}

        nc.gpsimd.wait_ge(s_dve, 2)  # dest_i columns 0:32 written
        for c in range(C):
            if c in chunk_start:
                k = chunk_start[c]
                nc.gpsimd.wait_ge(s_xs, 16 * (k + 1))
                nc.gpsimd.wait_ge(s_xc, 16 * (k + 1))
            if c == C // 2:
                nc.gpsimd.wait_ge(s_dve, 3)  # dest_i columns 32:64
            _indirect_scatter_write(
                nc, out_d[:], dest_i[:, c:c + 1],
                xt[:, c * D:(c + 1) * D],
                qname[0]).then_inc(s_sq0, 16)
        nc.gpsimd.wait_ge(s_sq0, 16 * (1 + C))

    nc.compile()
    return nc


def _get_nc():
    global _cached
    if _cached is None:
        _cached = _build()
    return _cached


def _constants():
    cst_big = np.ascontiguousarray(
        np.triu(np.ones((P, P), np.float32), k=1))
    ones_r = np.ones((1, P), np.float32)
    cst = (np.arange(G, dtype=np.float32) * CAP - 1.0).reshape(1, G)
    cst_row = np.concatenate([ones_r, cst], axis=1)
    return cst_big, cst_row


def kernel(x, block_onehot, capacity):
    from concourse.bass_utils import run_bass_kernel_spmd

    x = np.ascontiguousarray(np.asarray(x, dtype=np.float32))
    oh = np.asarray(block_onehot, dtype=np.float32)
    if oh.ndim == 2:
        oh = np.broadcast_to(oh[None], (B,) + oh.shape)
    oh = np.ascontiguousarray(oh)
    assert x.shape == (B, N, D), x.shape
    assert oh.shape == (B, N, G), oh.shape
    assert int(capacity) == CAP, capacity
    nc = _get_nc()
    cst_big, cst_row = _constants()
    in_maps = [
        {"x": x[b], "oh": oh[b], "cst_big": cst_big, "cst_row": cst_row}
        for b in range(B)
    ]
    res = run_bass_kernel_spmd(nc, in_maps, core_ids=list(range(NCORES)))
    return np.stack([res.results[b]["out"].reshape(G, CAP, D)
                     for b in range(B)])

